# revision 3
# baseline (speedup 1.0000x reference)
"""TransformerConv (heads=1) + ELU — Bass/Tile kernel v3 on 8 NeuronCores.

Sharding strategy (1D partition by target node, per the halo-exchange hint):
  dst nodes are sharded 8 ways; during input sharding the host performs the
  halo exchange — each core receives the source-node feature rows its edges
  reference, laid out in edge-slot order (slot = (block, lane)), plus fp8
  one-hot routing matrices. All compute (QK logits, segment softmax,
  scatter-add aggregation, output projection, skip + ELU) runs on device:

  - logit_e = qk[dst_e] . x[src_e] with Wqk = scale*(Wq @ Wk^T) folded on
    host; qk = x@Wqk + bqk computed on device for the dst slice (phase 1).
  - pqg = one-hot-select(qk) per edge via fp8-lhsT x bf16-rhs PE matmuls.
  - prod/reduce/exp/mex in full-block batched DVE/Pool/Act ops.
  - aggregation via mex^T @ xg PE matmuls accumulating in PSUM; den via ones;
    Wv folded after aggregation (pagg transpose + single matmul per block).
  - z = agg/den + skip staged bf16; global ELU epilogue in wide strips.
"""
import math
import os
import numpy as np
import ml_dtypes

BF16 = ml_dtypes.bfloat16
FP8 = ml_dtypes.float8_e4m3fn

N, E, D = 100000, 800000, 128
M_CORES = 8
DPC = N // M_CORES                 # 12500
NB = (DPC + 127) // 128            # 98
DST_PAD = NB * 128                 # 12544
SCALE = 1.0 / math.sqrt(D)
CB = 9                             # chunk-columns per block (uniform)
S = NB * CB                        # 882 cols per core
GANG = 7                           # blocks per load group
NGANG = NB // GANG


def _host_prep(edge_index, x_bf):
    """Pack edges into per-block 128-slot columns; build halo-exchanged
    feature rows + fp8 one-hot routing tables per core."""
    src = np.asarray(edge_index[0], dtype=np.int64)
    dst = np.asarray(edge_index[1], dtype=np.int64)
    core = dst // DPC
    ld = dst - core * DPC
    cap = CB * 128

    cores = []
    for c in range(M_CORES):
        sel = core == c
        e_ld = ld[sel]
        e_src = src[sel]
        deg = np.bincount(e_ld, minlength=DPC)[:DPC]

        # batched LPT: dsts -> blocks, balancing edge counts, <=128 dsts/block
        order = np.argsort(-deg)
        loads = np.zeros(NB, np.int64)
        blk = np.zeros(DPC, np.int64)
        for k in range(0, DPC, NB):
            batch = order[k:k + NB]
            binord = np.argsort(loads)
            blk[batch] = binord[:len(batch)]
            loads[binord[:len(batch)]] += deg[batch]
        assert loads.max() <= cap, f"block overflow {loads.max()}"

        order2 = np.argsort(blk, kind="stable")
        blk_sorted = blk[order2]
        starts = np.searchsorted(blk_sorted, np.arange(NB))
        lane = np.arange(DPC) - starts[blk_sorted]
        assert lane.max() < 128
        perm = np.full(DST_PAD, DPC, np.int64)
        perm[blk_sorted * 128 + lane] = order2
        lane_of = np.zeros(DPC, np.int64)
        lane_of[order2] = lane

        # edge slots: per block, edges packed col-major into CB columns
        e_blk = blk[e_ld]
        g_order = np.argsort(e_blk, kind="stable")
        gb = e_blk[g_order]
        counts = np.bincount(gb, minlength=NB)
        estarts = np.concatenate([[0], np.cumsum(counts)[:-1]])
        j = np.arange(len(gb)) - estarts[gb]          # rank within block
        col = gb * CB + j // 128                      # global col
        pp = j % 128                                  # lane (partition)

        slot_src = np.zeros((128, S), np.int64)       # pad -> row 0
        dstloc = np.full((128, S), 255, np.int64)
        slot_src[pp, col] = e_src[g_order]
        dstloc[pp, col] = lane_of[e_ld[g_order]]

        # halo-exchanged feature rows + ones col, [128(lane), S, 132] bf16
        xg = np.zeros((128, S, 132), BF16)
        xg[:, :, 0:128] = x_bf[slot_src.reshape(-1)].reshape(128, S, 128)
        xg[:, :, 128] = 1.0

        # fp8 one-hots: oh8 [128(e), S, 128(d)], ohT8 [128(d), S, 128(e)]
        oh8 = np.zeros((128, S, 128), FP8)
        ohT8 = np.zeros((128, S, 128), FP8)
        ppn, ccn = np.nonzero(dstloc < 128)
        dd = dstloc[ppn, ccn]
        oh8[ppn, ccn, dd] = 1.0
        ohT8[dd, ccn, ppn] = 1.0

        cores.append({"xg": xg, "oh8": oh8, "ohT8": ohT8, "perm": perm})
    return cores


def _build_nc():
    from contextlib import ExitStack
    import concourse.tile as tile
    from concourse import bacc, mybir

    fp32 = mybir.dt.float32
    bf16 = mybir.dt.bfloat16
    fp8 = mybir.dt.float8e4
    Alu = mybir.AluOpType
    Act = mybir.ActivationFunctionType

    nc = bacc.Bacc("TRN2", target_bir_lowering=False, debug=False)

    xg_d = nc.dram_tensor("xg", [128, S * 132], bf16, kind="ExternalInput").ap()
    oh8_d = nc.dram_tensor("oh8", [128, S * 128], fp8, kind="ExternalInput").ap()
    ohT8_d = nc.dram_tensor("ohT8", [128, S * 128], fp8, kind="ExternalInput").ap()
    xTs = nc.dram_tensor("xTs", [128, DST_PAD], bf16, kind="ExternalInput").ap()
    Wqk = nc.dram_tensor("Wqk", [128, 128], bf16, kind="ExternalInput").ap()
    Ws = nc.dram_tensor("Ws", [128, 128], bf16, kind="ExternalInput").ap()
    Wv = nc.dram_tensor("Wv", [128, 128], bf16, kind="ExternalInput").ap()
    bqk1 = nc.dram_tensor("bqk1", [1, 128], bf16, kind="ExternalInput").ap()
    bsv1 = nc.dram_tensor("bsv1", [1, 128], bf16, kind="ExternalInput").ap()
    ident_d = nc.dram_tensor("ident", [128, 128], bf16, kind="ExternalInput").ap()
    out_d = nc.dram_tensor("out", [DST_PAD, 128], bf16, kind="ExternalOutput").ap()

    with tile.TileContext(nc) as tc, ExitStack() as ctx:
        const_p = ctx.enter_context(tc.tile_pool(name="const", bufs=1))

        w_qk = const_p.tile([128, 128], bf16, tag="wqk")
        w_s = const_p.tile([128, 128], bf16, tag="ws")
        w_v = const_p.tile([128, 128], bf16, tag="wv")
        b_qk = const_p.tile([1, 128], bf16, tag="bqk")
        b_sv = const_p.tile([1, 128], bf16, tag="bsv")
        nc.sync.dma_start(w_qk[:], Wqk[:])
        nc.sync.dma_start(w_s[:], Ws[:])
        nc.sync.dma_start(w_v[:], Wv[:])
        nc.sync.dma_start(b_qk[:], bqk1[:])
        nc.sync.dma_start(b_sv[:], bsv1[:])
        ones1 = const_p.tile([1, 128], bf16, tag="ones1")
        nc.vector.memset(ones1[:], 1.0)
        ones_col = const_p.tile([128, 1], bf16, tag="ones_col")
        nc.vector.memset(ones_col[:], 1.0)
        ident = const_p.tile([128, 128], bf16, tag="ident")
        nc.sync.dma_start(ident[:], ident_d[:])

        qk_sb = const_p.tile([128, NB, 128], bf16, tag="qksb")
        skip_sb = const_p.tile([128, NB, 128], bf16, tag="skipsb")
        z_sb = const_p.tile([128, NB, 128], bf16, tag="zsb")

        # ---------------- phase 1: qk + skip for the dst slice ----------------
        TW = 2048
        tiles1 = [(i * TW, TW) for i in range(DST_PAD // TW)]
        if DST_PAD % TW:
            tiles1.append((DST_PAD // TW * TW, DST_PAD % TW))
        with tc.tile_pool(name="p1x", bufs=3) as p1x, \
             tc.tile_pool(name="p1ps", bufs=4, space="PSUM") as p1ps:
            for (base, w) in tiles1:
                nj = w // 128
                xt = p1x.tile([128, w], bf16, tag="xst")
                nc.sync.dma_start(xt[:], xTs[:, base:base + w])
                for j0 in range(0, nj, 4):
                    js = list(range(j0, min(j0 + 4, nj)))
                    gn = len(js)
                    pq = p1ps.tile([128, gn, 128], fp32, tag="pq")
                    ps = p1ps.tile([128, gn, 128], fp32, tag="ps")
                    for i, jj in enumerate(js):
                        lhs = xt[:, jj * 128:(jj + 1) * 128]
                        nc.tensor.matmul(out=pq[:, i, :], lhsT=lhs, rhs=w_qk[:],
                                         start=True, stop=False)
                        nc.tensor.matmul(out=pq[:, i, :], lhsT=ones1[:], rhs=b_qk[:],
                                         start=False, stop=True)
                        nc.tensor.matmul(out=ps[:, i, :], lhsT=lhs, rhs=w_s[:],
                                         start=True, stop=False)
                        nc.tensor.matmul(out=ps[:, i, :], lhsT=ones1[:], rhs=b_sv[:],
                                         start=False, stop=True)
                    blk0 = base // 128 + j0
                    nc.scalar.activation(qk_sb[:, blk0:blk0 + gn, :], pq[:], Act.Copy)
                    nc.vector.tensor_copy(skip_sb[:, blk0:blk0 + gn, :], ps[:])

        # ---------------- phase 2: edge attention + scatter ----------------
        with tc.tile_pool(name="gxg", bufs=2) as gxg_p, \
             tc.tile_pool(name="goh", bufs=2) as goh_p, \
             tc.tile_pool(name="goht", bufs=2) as goht_p, \
             tc.tile_pool(name="prod", bufs=3) as prod_p, \
             tc.tile_pool(name="mexp", bufs=3) as mex_p, \
             tc.tile_pool(name="smal", bufs=8) as small_p, \
             tc.tile_pool(name="tails", bufs=4) as tails_p, \
             tc.tile_pool(name="pqps", bufs=2, space="PSUM") as pq_ps, \
             tc.tile_pool(name="pags", bufs=2, space="PSUM") as pag_ps, \
             tc.tile_pool(name="tlps", bufs=1, space="PSUM") as tail_ps:
            for g in range(NGANG):
                S_g = GANG * CB
                cb = g * GANG * CB
                xgt = gxg_p.tile([128, S_g, 132], bf16, tag="xg")
                nc.sync.dma_start(
                    xgt[:], xg_d[:, cb * 132:(cb + S_g) * 132]
                    .rearrange("p (s e) -> p s e", e=132))
                oh8 = goh_p.tile([128, S_g, 128], fp8, tag="oh8")
                nc.sync.dma_start(
                    oh8[:], oh8_d[:, cb * 128:(cb + S_g) * 128]
                    .rearrange("p (s e) -> p s e", e=128))
                ohT8 = goht_p.tile([128, S_g, 128], fp8, tag="ohT8")
                nc.sync.dma_start(
                    ohT8[:], ohT8_d[:, cb * 128:(cb + S_g) * 128]
                    .rearrange("p (s e) -> p s e", e=128))

                for b in range(g * GANG, (g + 1) * GANG):
                    lb = (b - g * GANG) * CB          # gang-local col base
                    pqg_sb = prod_p.tile([128, CB, 128], bf16, tag="pqgsb")
                    for h, (h0, hn) in enumerate([(0, 5), (5, 4)]):
                        pqg = pq_ps.tile([128, hn, 128], fp32, tag="pqg")
                        for i in range(hn):
                            nc.tensor.matmul(out=pqg[:, i, :],
                                             lhsT=ohT8[:, lb + h0 + i, :],
                                             rhs=qk_sb[:, b, :],
                                             start=True, stop=True)
                        nc.scalar.activation(pqg_sb[:, h0:h0 + hn, :], pqg[:],
                                             Act.Copy)
                    # prod = pqg * xg  (bf16), logits = reduce; alternate
                    # prod/mex engines by block parity to balance DVE/Pool
                    prod_eng = nc.vector if b % 2 == 0 else nc.gpsimd
                    mex_eng = nc.gpsimd if b % 2 == 0 else nc.vector
                    prod = prod_p.tile([128, CB, 128], bf16, tag="prod")
                    prod_eng.tensor_tensor(
                        out=prod[:], in0=pqg_sb[:],
                        in1=xgt[:, lb:lb + CB, 0:128], op=Alu.mult)
                    logit = small_p.tile([128, CB], fp32, tag="logit")
                    nc.vector.tensor_reduce(
                        out=logit[:], in_=prod[:], op=Alu.add,
                        axis=mybir.AxisListType.X)
                    ex = small_p.tile([128, CB], fp32, tag="ex")
                    nc.scalar.activation(ex[:], logit[:], Act.Exp)
                    # mex = oh8 * ex (broadcast along d)
                    mex = mex_p.tile([128, CB, 128], bf16, tag="mex")
                    mex_eng.tensor_tensor(
                        out=mex[:], in0=oh8[:, lb:lb + CB, :],
                        in1=ex[:].rearrange("p (a one) -> p a one", one=1)
                        .to_broadcast([128, CB, 128]),
                        op=Alu.mult)
                    pagd = pag_ps.tile([128, 132], fp32, tag="pagd")
                    for i in range(CB):
                        nc.tensor.matmul(out=pagd[:], lhsT=mex[:, i, :],
                                         rhs=xgt[:, lb + i, :],
                                         start=(i == 0), stop=(i == CB - 1))
                    den = small_p.tile([128, 1], fp32, tag="den")
                    nc.vector.tensor_scalar_add(den[:], pagd[:, 128:129], 1e-30)
                    rec = small_p.tile([128, 1], fp32, tag="rec")
                    nc.vector.reciprocal(rec[:], den[:])
                    pagg_sb = tails_p.tile([128, 128], bf16, tag="paggsb")
                    nc.scalar.activation(pagg_sb[:], pagd[:, 0:128], Act.Copy)
                    paggT = tail_ps.tile([128, 128], bf16, tag="paggT")
                    nc.tensor.transpose(out=paggT[:], in_=pagg_sb[:], identity=ident[:])
                    paggT_sb = tails_p.tile([128, 128], bf16, tag="paggTsb")
                    nc.scalar.activation(paggT_sb[:], paggT[:], Act.Copy)
                    aggv = tail_ps.tile([128, 128], fp32, tag="aggv")
                    nc.tensor.matmul(out=aggv[:], lhsT=paggT_sb[:], rhs=w_v[:],
                                     start=True, stop=True)
                    nc.vector.scalar_tensor_tensor(
                        out=z_sb[:, b, :], in0=aggv[:], scalar=rec[:, 0:1],
                        in1=skip_sb[:, b, :], op0=Alu.mult, op1=Alu.add)

        # ---------------- phase 3: global ELU + store ----------------
        SB = 14
        with tc.tile_pool(name="epi", bufs=3) as epi_p:
            for b0 in range(0, NB, SB):
                zs = z_sb[:, b0:b0 + SB, :]
                e = epi_p.tile([128, SB, 128], bf16, tag="e")
                nc.scalar.activation(e[:], zs, Act.Exp)
                em = epi_p.tile([128, SB, 128], bf16, tag="em")
                nc.vector.tensor_scalar_add(em[:], e[:], -1.0)
                o = epi_p.tile([128, SB, 128], bf16, tag="o")
                nc.vector.scalar_tensor_tensor(
                    out=o[:], in0=zs, scalar=0.0, in1=em[:],
                    op0=Alu.max, op1=Alu.min)
                out_view = out_d[b0 * 128:(b0 + SB) * 128, :].rearrange(
                    "(j p) e -> p j e", p=128)
                nc.sync.dma_start(out_view, o[:])

    nc.compile()
    return nc


_NC_CACHE = {}


def _get_nc():
    if "nc" not in _NC_CACHE:
        _NC_CACHE["nc"] = _build_nc()
    return _NC_CACHE["nc"]


def _make_in_maps(inputs, cores):
    x = np.asarray(inputs["x"], np.float32)
    xb = x.astype(BF16)
    wq = np.asarray(inputs["Wq"], np.float32)
    wk = np.asarray(inputs["Wk"], np.float32)
    Wqk = (SCALE * (wq @ wk.T)).astype(BF16)
    bqk = (SCALE * (np.asarray(inputs["bq"], np.float32) @ wk.T)).astype(BF16)
    ws = np.asarray(inputs["Ws"], np.float32).astype(BF16)
    wv = np.asarray(inputs["Wv"], np.float32).astype(BF16)
    bsv = (np.asarray(inputs["bs"], np.float32)
           + np.asarray(inputs["bv"], np.float32)).astype(BF16)
    ident = np.eye(128, dtype=np.float32).astype(BF16)

    in_maps = []
    for c in range(M_CORES):
        co = cores[c]
        xs_local = np.zeros((DST_PAD, 128), BF16)
        xs_local[:DPC] = xb[c * DPC:(c + 1) * DPC]
        xTs = xs_local[np.minimum(co["perm"], DPC)].T.copy()
        in_maps.append({
            "xg": co["xg"].reshape(128, -1),
            "oh8": co["oh8"].reshape(128, -1),
            "ohT8": co["ohT8"].reshape(128, -1),
            "xTs": xTs,
            "Wqk": Wqk, "Ws": ws, "Wv": wv,
            "bqk1": bqk.reshape(1, 128), "bsv1": bsv.reshape(1, 128),
            "ident": ident,
        })
    return in_maps


def kernel(x, edge_index, Wq, bq, Wk, bk, Wv, bv, Ws, bs):
    from concourse import bass_utils

    xb = np.asarray(x, np.float32).astype(BF16)
    cores = _host_prep(edge_index, xb)
    in_maps = _make_in_maps(
        {"x": x, "Wq": Wq, "Wk": Wk, "Wv": Wv, "Ws": Ws,
         "bq": bq, "bs": bs, "bv": bv}, cores)
    nc = _get_nc()
    res = bass_utils.run_bass_kernel_spmd(nc, in_maps, core_ids=list(range(M_CORES)))
    out = np.zeros((N, 128), np.float32)
    for c in range(M_CORES):
        rows = res.results[c]["out"].astype(np.float32)
        p = cores[c]["perm"]
        valid = p < DPC
        out[c * DPC + p[valid]] = rows[valid]
    return out


# revision 4
# speedup vs baseline: 1.1130x; 1.1130x over previous
"""TransformerConv (heads=1) + ELU — Bass/Tile kernel v3 on 8 NeuronCores.

Sharding strategy (1D partition by target node, per the halo-exchange hint):
  dst nodes are sharded 8 ways; during input sharding the host performs the
  halo exchange — each core receives the source-node feature rows its edges
  reference, laid out in edge-slot order (slot = (block, lane)), plus fp8
  one-hot routing matrices. All compute (QK logits, segment softmax,
  scatter-add aggregation, output projection, skip + ELU) runs on device:

  - logit_e = qk[dst_e] . x[src_e] with Wqk = scale*(Wq @ Wk^T) folded on
    host; qk = x@Wqk + bqk computed on device for the dst slice (phase 1).
  - pqg = one-hot-select(qk) per edge via fp8-lhsT x bf16-rhs PE matmuls.
  - prod/reduce/exp/mex in full-block batched DVE/Pool/Act ops.
  - aggregation via mex^T @ xg PE matmuls accumulating in PSUM; den via ones;
    Wv folded after aggregation (pagg transpose + single matmul per block).
  - z = agg/den + skip staged bf16; global ELU epilogue in wide strips.
"""
import math
import os
import numpy as np
import ml_dtypes

BF16 = ml_dtypes.bfloat16
FP8 = ml_dtypes.float8_e4m3fn

N, E, D = 100000, 800000, 128
M_CORES = 8
DPC = N // M_CORES                 # 12500
NB = (DPC + 127) // 128            # 98
DST_PAD = NB * 128                 # 12544
SCALE = 1.0 / math.sqrt(D)
CB = 9                             # chunk-columns per block (uniform)
S = NB * CB                        # 882 cols per core
GANG = 7                           # blocks per load group
NGANG = NB // GANG


def _host_prep(edge_index, x_bf):
    """Pack edges into per-block 128-slot columns; build halo-exchanged
    feature rows + fp8 one-hot routing tables per core."""
    src = np.asarray(edge_index[0], dtype=np.int64)
    dst = np.asarray(edge_index[1], dtype=np.int64)
    core = dst // DPC
    ld = dst - core * DPC
    cap = CB * 128

    cores = []
    for c in range(M_CORES):
        sel = core == c
        e_ld = ld[sel]
        e_src = src[sel]
        deg = np.bincount(e_ld, minlength=DPC)[:DPC]

        # batched LPT: dsts -> blocks, balancing edge counts, <=128 dsts/block
        order = np.argsort(-deg)
        loads = np.zeros(NB, np.int64)
        blk = np.zeros(DPC, np.int64)
        for k in range(0, DPC, NB):
            batch = order[k:k + NB]
            binord = np.argsort(loads)
            blk[batch] = binord[:len(batch)]
            loads[binord[:len(batch)]] += deg[batch]
        assert loads.max() <= cap, f"block overflow {loads.max()}"

        order2 = np.argsort(blk, kind="stable")
        blk_sorted = blk[order2]
        starts = np.searchsorted(blk_sorted, np.arange(NB))
        lane = np.arange(DPC) - starts[blk_sorted]
        assert lane.max() < 128
        perm = np.full(DST_PAD, DPC, np.int64)
        perm[blk_sorted * 128 + lane] = order2
        lane_of = np.zeros(DPC, np.int64)
        lane_of[order2] = lane

        # edge slots: per block, edges packed col-major into CB columns
        e_blk = blk[e_ld]
        g_order = np.argsort(e_blk, kind="stable")
        gb = e_blk[g_order]
        counts = np.bincount(gb, minlength=NB)
        estarts = np.concatenate([[0], np.cumsum(counts)[:-1]])
        j = np.arange(len(gb)) - estarts[gb]          # rank within block
        col = gb * CB + j // 128                      # global col
        pp = j % 128                                  # lane (partition)

        slot_src = np.zeros((128, S), np.int64)       # pad -> row 0
        dstloc = np.full((128, S), 255, np.int64)
        slot_src[pp, col] = e_src[g_order]
        dstloc[pp, col] = lane_of[e_ld[g_order]]

        # halo-exchanged feature rows + ones col, [128(lane), S, 132] bf16
        xg = np.zeros((128, S, 132), BF16)
        xg[:, :, 0:128] = x_bf[slot_src.reshape(-1)].reshape(128, S, 128)
        xg[:, :, 128] = 1.0

        # fp8 one-hots: oh8 [128(e), S, 128(d)], ohT8 [128(d), S, 128(e)]
        oh8 = np.zeros((128, S, 128), FP8)
        ohT8 = np.zeros((128, S, 128), FP8)
        ppn, ccn = np.nonzero(dstloc < 128)
        dd = dstloc[ppn, ccn]
        oh8[ppn, ccn, dd] = 1.0
        ohT8[dd, ccn, ppn] = 1.0

        cores.append({"xg": xg, "oh8": oh8, "ohT8": ohT8, "perm": perm})
    return cores


def _build_nc():
    from contextlib import ExitStack
    import concourse.tile as tile
    from concourse import bacc, mybir

    fp32 = mybir.dt.float32
    bf16 = mybir.dt.bfloat16
    fp8 = mybir.dt.float8e4
    Alu = mybir.AluOpType
    Act = mybir.ActivationFunctionType

    nc = bacc.Bacc("TRN2", target_bir_lowering=False, debug=False)

    xg_d = nc.dram_tensor("xg", [128, S * 132], bf16, kind="ExternalInput").ap()
    oh8_d = nc.dram_tensor("oh8", [128, S * 128], fp8, kind="ExternalInput").ap()
    ohT8_d = nc.dram_tensor("ohT8", [128, S * 128], fp8, kind="ExternalInput").ap()
    xTs = nc.dram_tensor("xTs", [128, DST_PAD], bf16, kind="ExternalInput").ap()
    Wqk = nc.dram_tensor("Wqk", [128, 128], bf16, kind="ExternalInput").ap()
    Ws = nc.dram_tensor("Ws", [128, 128], bf16, kind="ExternalInput").ap()
    Wv = nc.dram_tensor("Wv", [128, 128], bf16, kind="ExternalInput").ap()
    bqk1 = nc.dram_tensor("bqk1", [1, 128], bf16, kind="ExternalInput").ap()
    bsv1 = nc.dram_tensor("bsv1", [1, 128], bf16, kind="ExternalInput").ap()
    ident_d = nc.dram_tensor("ident", [128, 128], bf16, kind="ExternalInput").ap()
    out_d = nc.dram_tensor("out", [DST_PAD, 128], bf16, kind="ExternalOutput").ap()

    with tile.TileContext(nc) as tc, ExitStack() as ctx:
        const_p = ctx.enter_context(tc.tile_pool(name="const", bufs=1))

        w_qk = const_p.tile([128, 128], bf16, tag="wqk")
        w_s = const_p.tile([128, 128], bf16, tag="ws")
        w_v = const_p.tile([128, 128], bf16, tag="wv")
        b_qk = const_p.tile([1, 128], bf16, tag="bqk")
        b_sv = const_p.tile([1, 128], bf16, tag="bsv")
        nc.sync.dma_start(w_qk[:], Wqk[:])
        nc.sync.dma_start(w_s[:], Ws[:])
        nc.sync.dma_start(w_v[:], Wv[:])
        nc.sync.dma_start(b_qk[:], bqk1[:])
        nc.sync.dma_start(b_sv[:], bsv1[:])
        ones1 = const_p.tile([1, 128], bf16, tag="ones1")
        nc.vector.memset(ones1[:], 1.0)
        ones_col = const_p.tile([128, 1], bf16, tag="ones_col")
        nc.vector.memset(ones_col[:], 1.0)
        ident = const_p.tile([128, 128], bf16, tag="ident")
        nc.sync.dma_start(ident[:], ident_d[:])

        qk_sb = const_p.tile([128, NB, 128], bf16, tag="qksb")
        skip_sb = const_p.tile([128, NB, 128], bf16, tag="skipsb")
        z_sb = const_p.tile([128, NB, 128], bf16, tag="zsb")

        # ---------------- phase 1: qk + skip for the dst slice ----------------
        TW = 2048
        tiles1 = [(i * TW, TW) for i in range(DST_PAD // TW)]
        if DST_PAD % TW:
            tiles1.append((DST_PAD // TW * TW, DST_PAD % TW))
        with tc.tile_pool(name="p1x", bufs=3) as p1x, \
             tc.tile_pool(name="p1ps", bufs=4, space="PSUM") as p1ps:
            for (base, w) in tiles1:
                nj = w // 128
                xt = p1x.tile([128, w], bf16, tag="xst")
                nc.sync.dma_start(xt[:], xTs[:, base:base + w])
                for j0 in range(0, nj, 4):
                    js = list(range(j0, min(j0 + 4, nj)))
                    gn = len(js)
                    pq = p1ps.tile([128, gn, 128], fp32, tag="pq")
                    ps = p1ps.tile([128, gn, 128], fp32, tag="ps")
                    for i, jj in enumerate(js):
                        lhs = xt[:, jj * 128:(jj + 1) * 128]
                        nc.tensor.matmul(out=pq[:, i, :], lhsT=lhs, rhs=w_qk[:],
                                         start=True, stop=False)
                        nc.tensor.matmul(out=pq[:, i, :], lhsT=ones1[:], rhs=b_qk[:],
                                         start=False, stop=True)
                        nc.tensor.matmul(out=ps[:, i, :], lhsT=lhs, rhs=w_s[:],
                                         start=True, stop=False)
                        nc.tensor.matmul(out=ps[:, i, :], lhsT=ones1[:], rhs=b_sv[:],
                                         start=False, stop=True)
                    blk0 = base // 128 + j0
                    nc.scalar.activation(qk_sb[:, blk0:blk0 + gn, :], pq[:], Act.Copy)
                    nc.vector.tensor_copy(skip_sb[:, blk0:blk0 + gn, :], ps[:])

        # ---------------- phase 2: edge attention + scatter ----------------
        with tc.tile_pool(name="gxg", bufs=2) as gxg_p, \
             tc.tile_pool(name="goh", bufs=2) as goh_p, \
             tc.tile_pool(name="goht", bufs=2) as goht_p, \
             tc.tile_pool(name="prod", bufs=3) as prod_p, \
             tc.tile_pool(name="mexp", bufs=3) as mex_p, \
             tc.tile_pool(name="smal", bufs=8) as small_p, \
             tc.tile_pool(name="tails", bufs=4) as tails_p, \
             tc.tile_pool(name="pqps", bufs=2, space="PSUM") as pq_ps, \
             tc.tile_pool(name="pags", bufs=2, space="PSUM") as pag_ps, \
             tc.tile_pool(name="tlps", bufs=1, space="PSUM") as tail_ps:
            for g in range(NGANG):
                S_g = GANG * CB
                cb = g * GANG * CB
                xgt = gxg_p.tile([128, S_g, 132], bf16, tag="xg")
                nc.sync.dma_start(
                    xgt[:], xg_d[:, cb * 132:(cb + S_g) * 132]
                    .rearrange("p (s e) -> p s e", e=132))
                oh8 = goh_p.tile([128, S_g, 128], fp8, tag="oh8")
                nc.sync.dma_start(
                    oh8[:], oh8_d[:, cb * 128:(cb + S_g) * 128]
                    .rearrange("p (s e) -> p s e", e=128))
                ohT8 = goht_p.tile([128, S_g, 128], fp8, tag="ohT8")
                nc.sync.dma_start(
                    ohT8[:], ohT8_d[:, cb * 128:(cb + S_g) * 128]
                    .rearrange("p (s e) -> p s e", e=128))

                for b in range(g * GANG, (g + 1) * GANG):
                    lb = (b - g * GANG) * CB          # gang-local col base
                    pqg_sb = prod_p.tile([128, CB, 128], bf16, tag="pqgsb")
                    for h, (h0, hn) in enumerate([(0, 5), (5, 4)]):
                        pqg = pq_ps.tile([128, hn, 128], fp32, tag="pqg")
                        for i in range(hn):
                            nc.tensor.matmul(out=pqg[:, i, :],
                                             lhsT=ohT8[:, lb + h0 + i, :],
                                             rhs=qk_sb[:, b, :],
                                             start=True, stop=True)
                        nc.scalar.activation(pqg_sb[:, h0:h0 + hn, :], pqg[:],
                                             Act.Copy)
                    # prod = pqg * xg  (bf16), logits = reduce over feat
                    prod = prod_p.tile([128, CB, 128], bf16, tag="prod")
                    nc.vector.tensor_tensor(
                        out=prod[:], in0=pqg_sb[:],
                        in1=xgt[:, lb:lb + CB, 0:128], op=Alu.mult)
                    logit = small_p.tile([128, CB], fp32, tag="logit")
                    nc.vector.tensor_reduce(
                        out=logit[:], in_=prod[:], op=Alu.add,
                        axis=mybir.AxisListType.X)
                    ex = small_p.tile([128, CB], fp32, tag="ex")
                    nc.scalar.activation(ex[:], logit[:], Act.Exp)
                    # mex = oh8 * ex (broadcast along d)
                    mex = mex_p.tile([128, CB, 128], bf16, tag="mex")
                    nc.gpsimd.tensor_tensor(
                        out=mex[:], in0=oh8[:, lb:lb + CB, :],
                        in1=ex[:].rearrange("p (a one) -> p a one", one=1)
                        .to_broadcast([128, CB, 128]),
                        op=Alu.mult)
                    pagd = pag_ps.tile([128, 132], fp32, tag="pagd")
                    for i in range(CB):
                        nc.tensor.matmul(out=pagd[:], lhsT=mex[:, i, :],
                                         rhs=xgt[:, lb + i, :],
                                         start=(i == 0), stop=(i == CB - 1))
                    den = small_p.tile([128, 1], fp32, tag="den")
                    nc.vector.tensor_scalar_add(den[:], pagd[:, 128:129], 1e-30)
                    rec = small_p.tile([128, 1], fp32, tag="rec")
                    nc.vector.reciprocal(rec[:], den[:])
                    pagg_sb = tails_p.tile([128, 128], bf16, tag="paggsb")
                    nc.scalar.activation(pagg_sb[:], pagd[:, 0:128], Act.Copy)
                    paggT = tail_ps.tile([128, 128], bf16, tag="paggT")
                    nc.tensor.transpose(out=paggT[:], in_=pagg_sb[:], identity=ident[:])
                    paggT_sb = tails_p.tile([128, 128], bf16, tag="paggTsb")
                    nc.scalar.activation(paggT_sb[:], paggT[:], Act.Copy)
                    aggv = tail_ps.tile([128, 128], fp32, tag="aggv")
                    nc.tensor.matmul(out=aggv[:], lhsT=paggT_sb[:], rhs=w_v[:],
                                     start=True, stop=True)
                    nc.vector.scalar_tensor_tensor(
                        out=z_sb[:, b, :], in0=aggv[:], scalar=rec[:, 0:1],
                        in1=skip_sb[:, b, :], op0=Alu.mult, op1=Alu.add)

        # ---------------- phase 3: global ELU + store ----------------
        SB = 14
        with tc.tile_pool(name="epi", bufs=3) as epi_p:
            for b0 in range(0, NB, SB):
                zs = z_sb[:, b0:b0 + SB, :]
                e = epi_p.tile([128, SB, 128], bf16, tag="e")
                nc.scalar.activation(e[:], zs, Act.Exp)
                em = epi_p.tile([128, SB, 128], bf16, tag="em")
                nc.vector.tensor_scalar_add(em[:], e[:], -1.0)
                o = epi_p.tile([128, SB, 128], bf16, tag="o")
                nc.vector.scalar_tensor_tensor(
                    out=o[:], in0=zs, scalar=0.0, in1=em[:],
                    op0=Alu.max, op1=Alu.min)
                out_view = out_d[b0 * 128:(b0 + SB) * 128, :].rearrange(
                    "(j p) e -> p j e", p=128)
                nc.sync.dma_start(out_view, o[:])

    nc.compile()
    return nc


_NC_CACHE = {}


def _get_nc():
    if "nc" not in _NC_CACHE:
        _NC_CACHE["nc"] = _build_nc()
    return _NC_CACHE["nc"]


def _make_in_maps(inputs, cores):
    x = np.asarray(inputs["x"], np.float32)
    xb = x.astype(BF16)
    wq = np.asarray(inputs["Wq"], np.float32)
    wk = np.asarray(inputs["Wk"], np.float32)
    Wqk = (SCALE * (wq @ wk.T)).astype(BF16)
    bqk = (SCALE * (np.asarray(inputs["bq"], np.float32) @ wk.T)).astype(BF16)
    ws = np.asarray(inputs["Ws"], np.float32).astype(BF16)
    wv = np.asarray(inputs["Wv"], np.float32).astype(BF16)
    bsv = (np.asarray(inputs["bs"], np.float32)
           + np.asarray(inputs["bv"], np.float32)).astype(BF16)
    ident = np.eye(128, dtype=np.float32).astype(BF16)

    in_maps = []
    for c in range(M_CORES):
        co = cores[c]
        xs_local = np.zeros((DST_PAD, 128), BF16)
        xs_local[:DPC] = xb[c * DPC:(c + 1) * DPC]
        xTs = xs_local[np.minimum(co["perm"], DPC)].T.copy()
        in_maps.append({
            "xg": co["xg"].reshape(128, -1),
            "oh8": co["oh8"].reshape(128, -1),
            "ohT8": co["ohT8"].reshape(128, -1),
            "xTs": xTs,
            "Wqk": Wqk, "Ws": ws, "Wv": wv,
            "bqk1": bqk.reshape(1, 128), "bsv1": bsv.reshape(1, 128),
            "ident": ident,
        })
    return in_maps


def kernel(x, edge_index, Wq, bq, Wk, bk, Wv, bv, Ws, bs):
    from concourse import bass_utils

    xb = np.asarray(x, np.float32).astype(BF16)
    cores = _host_prep(edge_index, xb)
    in_maps = _make_in_maps(
        {"x": x, "Wq": Wq, "Wk": Wk, "Wv": Wv, "Ws": Ws,
         "bq": bq, "bs": bs, "bv": bv}, cores)
    nc = _get_nc()
    res = bass_utils.run_bass_kernel_spmd(nc, in_maps, core_ids=list(range(M_CORES)))
    out = np.zeros((N, 128), np.float32)
    for c in range(M_CORES):
        rows = res.results[c]["out"].astype(np.float32)
        p = cores[c]["perm"]
        valid = p < DPC
        out[c * DPC + p[valid]] = rows[valid]
    return out


# revision 5
# speedup vs baseline: 1.1387x; 1.0231x over previous
"""TransformerConv (heads=1) + ELU — Bass/Tile kernel v3 on 8 NeuronCores.

Sharding strategy (1D partition by target node, per the halo-exchange hint):
  dst nodes are sharded 8 ways; during input sharding the host performs the
  halo exchange — each core receives the source-node feature rows its edges
  reference, laid out in edge-slot order (slot = (block, lane)), plus fp8
  one-hot routing matrices. All compute (QK logits, segment softmax,
  scatter-add aggregation, output projection, skip + ELU) runs on device:

  - logit_e = qk[dst_e] . x[src_e] with Wqk = scale*(Wq @ Wk^T) folded on
    host; qk = x@Wqk + bqk computed on device for the dst slice (phase 1).
  - pqg = one-hot-select(qk) per edge via fp8-lhsT x bf16-rhs PE matmuls.
  - prod/reduce/exp/mex in full-block batched DVE/Pool/Act ops.
  - aggregation via mex^T @ xg PE matmuls accumulating in PSUM; den via ones;
    Wv folded after aggregation (pagg transpose + single matmul per block).
  - z = agg/den + skip staged bf16; global ELU epilogue in wide strips.
"""
import math
import os
import numpy as np
import ml_dtypes

BF16 = ml_dtypes.bfloat16
FP8 = ml_dtypes.float8_e4m3fn

N, E, D = 100000, 800000, 128
M_CORES = 8
DPC = N // M_CORES                 # 12500
NB = (DPC + 127) // 128            # 98
DST_PAD = NB * 128                 # 12544
SCALE = 1.0 / math.sqrt(D)
CB = 9                             # chunk-columns per block (uniform)
S = NB * CB                        # 882 cols per core
GANG = 7                           # blocks per load group
NGANG = NB // GANG


def _host_prep(edge_index, x_bf):
    """Pack edges into per-block 128-slot columns; build halo-exchanged
    feature rows + fp8 one-hot routing tables per core."""
    src = np.asarray(edge_index[0], dtype=np.int64)
    dst = np.asarray(edge_index[1], dtype=np.int64)
    core = dst // DPC
    ld = dst - core * DPC
    cap = CB * 128

    cores = []
    for c in range(M_CORES):
        sel = core == c
        e_ld = ld[sel]
        e_src = src[sel]
        deg = np.bincount(e_ld, minlength=DPC)[:DPC]

        # batched LPT: dsts -> blocks, balancing edge counts, <=128 dsts/block
        order = np.argsort(-deg)
        loads = np.zeros(NB, np.int64)
        blk = np.zeros(DPC, np.int64)
        for k in range(0, DPC, NB):
            batch = order[k:k + NB]
            binord = np.argsort(loads)
            blk[batch] = binord[:len(batch)]
            loads[binord[:len(batch)]] += deg[batch]
        assert loads.max() <= cap, f"block overflow {loads.max()}"

        order2 = np.argsort(blk, kind="stable")
        blk_sorted = blk[order2]
        starts = np.searchsorted(blk_sorted, np.arange(NB))
        lane = np.arange(DPC) - starts[blk_sorted]
        assert lane.max() < 128
        perm = np.full(DST_PAD, DPC, np.int64)
        perm[blk_sorted * 128 + lane] = order2
        lane_of = np.zeros(DPC, np.int64)
        lane_of[order2] = lane

        # edge slots: per block, edges packed col-major into CB columns
        e_blk = blk[e_ld]
        g_order = np.argsort(e_blk, kind="stable")
        gb = e_blk[g_order]
        counts = np.bincount(gb, minlength=NB)
        estarts = np.concatenate([[0], np.cumsum(counts)[:-1]])
        j = np.arange(len(gb)) - estarts[gb]          # rank within block
        col = gb * CB + j // 128                      # global col
        pp = j % 128                                  # lane (partition)

        slot_src = np.zeros((128, S), np.int64)       # pad -> row 0
        dstloc = np.full((128, S), 255, np.int64)
        slot_src[pp, col] = e_src[g_order]
        dstloc[pp, col] = lane_of[e_ld[g_order]]

        # halo-exchanged feature rows + ones col, [128(lane), S, 132] bf16
        xg = np.zeros((128, S, 132), BF16)
        xg[:, :, 0:128] = x_bf[slot_src.reshape(-1)].reshape(128, S, 128)
        xg[:, :, 128] = 1.0

        # fp8 one-hots: oh8 [128(e), S, 128(d)], ohT8 [128(d), S, 128(e)]
        oh8 = np.zeros((128, S, 128), FP8)
        ohT8 = np.zeros((128, S, 128), FP8)
        ppn, ccn = np.nonzero(dstloc < 128)
        dd = dstloc[ppn, ccn]
        oh8[ppn, ccn, dd] = 1.0
        ohT8[dd, ccn, ppn] = 1.0

        cores.append({"xg": xg, "oh8": oh8, "ohT8": ohT8, "perm": perm})
    return cores


def _build_nc():
    from contextlib import ExitStack
    import concourse.tile as tile
    from concourse import bacc, mybir

    fp32 = mybir.dt.float32
    bf16 = mybir.dt.bfloat16
    fp8 = mybir.dt.float8e4
    Alu = mybir.AluOpType
    Act = mybir.ActivationFunctionType

    nc = bacc.Bacc("TRN2", target_bir_lowering=False, debug=False)

    xg_d = nc.dram_tensor("xg", [128, S * 132], bf16, kind="ExternalInput").ap()
    oh8_d = nc.dram_tensor("oh8", [128, S * 128], fp8, kind="ExternalInput").ap()
    ohT8_d = nc.dram_tensor("ohT8", [128, S * 128], fp8, kind="ExternalInput").ap()
    xTs = nc.dram_tensor("xTs", [128, DST_PAD], bf16, kind="ExternalInput").ap()
    Wqk = nc.dram_tensor("Wqk", [128, 128], bf16, kind="ExternalInput").ap()
    Ws = nc.dram_tensor("Ws", [128, 128], bf16, kind="ExternalInput").ap()
    Wv = nc.dram_tensor("Wv", [128, 128], bf16, kind="ExternalInput").ap()
    bqk1 = nc.dram_tensor("bqk1", [1, 128], bf16, kind="ExternalInput").ap()
    bsv1 = nc.dram_tensor("bsv1", [1, 128], bf16, kind="ExternalInput").ap()
    ident_d = nc.dram_tensor("ident", [128, 128], bf16, kind="ExternalInput").ap()
    out_d = nc.dram_tensor("out", [DST_PAD, 128], bf16, kind="ExternalOutput").ap()

    with tile.TileContext(nc) as tc, ExitStack() as ctx:
        const_p = ctx.enter_context(tc.tile_pool(name="const", bufs=1))

        w_qk = const_p.tile([128, 128], bf16, tag="wqk")
        w_s = const_p.tile([128, 128], bf16, tag="ws")
        w_v = const_p.tile([128, 128], bf16, tag="wv")
        b_qk = const_p.tile([1, 128], bf16, tag="bqk")
        b_sv = const_p.tile([1, 128], bf16, tag="bsv")
        nc.sync.dma_start(w_qk[:], Wqk[:])
        nc.sync.dma_start(w_s[:], Ws[:])
        nc.sync.dma_start(w_v[:], Wv[:])
        nc.sync.dma_start(b_qk[:], bqk1[:])
        nc.sync.dma_start(b_sv[:], bsv1[:])
        ones1 = const_p.tile([1, 128], bf16, tag="ones1")
        nc.vector.memset(ones1[:], 1.0)
        ones_col = const_p.tile([128, 1], bf16, tag="ones_col")
        nc.vector.memset(ones_col[:], 1.0)
        ident = const_p.tile([128, 128], bf16, tag="ident")
        nc.sync.dma_start(ident[:], ident_d[:])

        qk_sb = const_p.tile([128, NB, 128], bf16, tag="qksb")
        skip_sb = const_p.tile([128, NB, 128], bf16, tag="skipsb")
        z_sb = const_p.tile([128, NB, 128], bf16, tag="zsb")

        # ---------------- phase 1: qk + skip for the dst slice ----------------
        TW = 2048
        tiles1 = [(i * TW, TW) for i in range(DST_PAD // TW)]
        if DST_PAD % TW:
            tiles1.append((DST_PAD // TW * TW, DST_PAD % TW))
        with tc.tile_pool(name="p1x", bufs=3) as p1x, \
             tc.tile_pool(name="p1ps", bufs=4, space="PSUM") as p1ps:
            for (base, w) in tiles1:
                nj = w // 128
                xt = p1x.tile([128, w], bf16, tag="xst")
                nc.sync.dma_start(xt[:], xTs[:, base:base + w])
                for j0 in range(0, nj, 4):
                    js = list(range(j0, min(j0 + 4, nj)))
                    gn = len(js)
                    pq = p1ps.tile([128, gn, 128], fp32, tag="pq")
                    ps = p1ps.tile([128, gn, 128], fp32, tag="ps")
                    for i, jj in enumerate(js):
                        lhs = xt[:, jj * 128:(jj + 1) * 128]
                        nc.tensor.matmul(out=pq[:, i, :], lhsT=lhs, rhs=w_qk[:],
                                         start=True, stop=False)
                        nc.tensor.matmul(out=pq[:, i, :], lhsT=ones1[:], rhs=b_qk[:],
                                         start=False, stop=True)
                        nc.tensor.matmul(out=ps[:, i, :], lhsT=lhs, rhs=w_s[:],
                                         start=True, stop=False)
                        nc.tensor.matmul(out=ps[:, i, :], lhsT=ones1[:], rhs=b_sv[:],
                                         start=False, stop=True)
                    blk0 = base // 128 + j0
                    nc.scalar.activation(qk_sb[:, blk0:blk0 + gn, :], pq[:], Act.Copy)
                    nc.vector.tensor_copy(skip_sb[:, blk0:blk0 + gn, :], ps[:])

        # ---------------- phase 2: edge attention + scatter ----------------
        with tc.tile_pool(name="gxg", bufs=2) as gxg_p, \
             tc.tile_pool(name="goh", bufs=2) as goh_p, \
             tc.tile_pool(name="goht", bufs=2) as goht_p, \
             tc.tile_pool(name="prod", bufs=3) as prod_p, \
             tc.tile_pool(name="mexp", bufs=3) as mex_p, \
             tc.tile_pool(name="smal", bufs=8) as small_p, \
             tc.tile_pool(name="tails", bufs=4) as tails_p, \
             tc.tile_pool(name="pqps", bufs=2, space="PSUM") as pq_ps, \
             tc.tile_pool(name="pags", bufs=1, space="PSUM") as pag_ps, \
             tc.tile_pool(name="tlps", bufs=2, space="PSUM") as tail_ps:
            for g in range(NGANG):
                S_g = GANG * CB
                cb = g * GANG * CB
                xgt = gxg_p.tile([128, S_g, 132], bf16, tag="xg")
                nc.sync.dma_start(
                    xgt[:], xg_d[:, cb * 132:(cb + S_g) * 132]
                    .rearrange("p (s e) -> p s e", e=132))
                oh8 = goh_p.tile([128, S_g, 128], fp8, tag="oh8")
                nc.sync.dma_start(
                    oh8[:], oh8_d[:, cb * 128:(cb + S_g) * 128]
                    .rearrange("p (s e) -> p s e", e=128))
                ohT8 = goht_p.tile([128, S_g, 128], fp8, tag="ohT8")
                nc.sync.dma_start(
                    ohT8[:], ohT8_d[:, cb * 128:(cb + S_g) * 128]
                    .rearrange("p (s e) -> p s e", e=128))

                for b in range(g * GANG, (g + 1) * GANG):
                    lb = (b - g * GANG) * CB          # gang-local col base
                    pqg_sb = prod_p.tile([128, CB, 128], bf16, tag="pqgsb")
                    for h, (h0, hn) in enumerate([(0, 5), (5, 4)]):
                        pqg = pq_ps.tile([128, hn, 128], fp32, tag="pqg")
                        for i in range(hn):
                            nc.tensor.matmul(out=pqg[:, i, :],
                                             lhsT=ohT8[:, lb + h0 + i, :],
                                             rhs=qk_sb[:, b, :],
                                             start=True, stop=True)
                        nc.scalar.activation(pqg_sb[:, h0:h0 + hn, :], pqg[:],
                                             Act.Copy)
                    # prod = pqg * xg  (bf16), logits = reduce over feat
                    prod = prod_p.tile([128, CB, 128], bf16, tag="prod")
                    nc.vector.tensor_tensor(
                        out=prod[:], in0=pqg_sb[:],
                        in1=xgt[:, lb:lb + CB, 0:128], op=Alu.mult)
                    logit = small_p.tile([128, CB], fp32, tag="logit")
                    nc.vector.tensor_reduce(
                        out=logit[:], in_=prod[:], op=Alu.add,
                        axis=mybir.AxisListType.X)
                    ex = small_p.tile([128, CB], fp32, tag="ex")
                    nc.scalar.activation(ex[:], logit[:], Act.Exp)
                    # mex = oh8 * ex (broadcast along d)
                    mex = mex_p.tile([128, CB, 128], bf16, tag="mex")
                    nc.gpsimd.tensor_tensor(
                        out=mex[:], in0=oh8[:, lb:lb + CB, :],
                        in1=ex[:].rearrange("p (a one) -> p a one", one=1)
                        .to_broadcast([128, CB, 128]),
                        op=Alu.mult)
                    pagd = pag_ps.tile([128, 132], fp32, tag="pagd")
                    for i in range(CB):
                        nc.tensor.matmul(out=pagd[:], lhsT=mex[:, i, :],
                                         rhs=xgt[:, lb + i, :],
                                         start=(i == 0), stop=(i == CB - 1))
                    den = small_p.tile([128, 1], fp32, tag="den")
                    nc.vector.tensor_scalar_add(den[:], pagd[:, 128:129], 1e-30)
                    rec = small_p.tile([128, 1], fp32, tag="rec")
                    nc.vector.reciprocal(rec[:], den[:])
                    pagg_sb = tails_p.tile([128, 128], bf16, tag="paggsb")
                    nc.scalar.activation(pagg_sb[:], pagd[:, 0:128], Act.Copy)
                    tail = tail_ps.tile([128, 192], fp32, tag="tail")
                    paggT = tail[:, 128:192].bitcast(bf16)
                    nc.tensor.transpose(out=paggT, in_=pagg_sb[:], identity=ident[:])
                    paggT_sb = tails_p.tile([128, 128], bf16, tag="paggTsb")
                    nc.scalar.activation(paggT_sb[:], paggT, Act.Copy)
                    aggv = tail[:, 0:128]
                    nc.tensor.matmul(out=aggv, lhsT=paggT_sb[:], rhs=w_v[:],
                                     start=True, stop=True)
                    nc.vector.scalar_tensor_tensor(
                        out=z_sb[:, b, :], in0=aggv, scalar=rec[:, 0:1],
                        in1=skip_sb[:, b, :], op0=Alu.mult, op1=Alu.add)

        # ---------------- phase 3: global ELU + store ----------------
        SB = 14
        with tc.tile_pool(name="epi", bufs=3) as epi_p:
            for b0 in range(0, NB, SB):
                zs = z_sb[:, b0:b0 + SB, :]
                e = epi_p.tile([128, SB, 128], bf16, tag="e")
                nc.scalar.activation(e[:], zs, Act.Exp)
                em = epi_p.tile([128, SB, 128], bf16, tag="em")
                nc.vector.tensor_scalar_add(em[:], e[:], -1.0)
                o = epi_p.tile([128, SB, 128], bf16, tag="o")
                nc.vector.scalar_tensor_tensor(
                    out=o[:], in0=zs, scalar=0.0, in1=em[:],
                    op0=Alu.max, op1=Alu.min)
                out_view = out_d[b0 * 128:(b0 + SB) * 128, :].rearrange(
                    "(j p) e -> p j e", p=128)
                nc.sync.dma_start(out_view, o[:])

    nc.compile()
    return nc


_NC_CACHE = {}


def _get_nc():
    if "nc" not in _NC_CACHE:
        _NC_CACHE["nc"] = _build_nc()
    return _NC_CACHE["nc"]


def _make_in_maps(inputs, cores):
    x = np.asarray(inputs["x"], np.float32)
    xb = x.astype(BF16)
    wq = np.asarray(inputs["Wq"], np.float32)
    wk = np.asarray(inputs["Wk"], np.float32)
    Wqk = (SCALE * (wq @ wk.T)).astype(BF16)
    bqk = (SCALE * (np.asarray(inputs["bq"], np.float32) @ wk.T)).astype(BF16)
    ws = np.asarray(inputs["Ws"], np.float32).astype(BF16)
    wv = np.asarray(inputs["Wv"], np.float32).astype(BF16)
    bsv = (np.asarray(inputs["bs"], np.float32)
           + np.asarray(inputs["bv"], np.float32)).astype(BF16)
    ident = np.eye(128, dtype=np.float32).astype(BF16)

    in_maps = []
    for c in range(M_CORES):
        co = cores[c]
        xs_local = np.zeros((DST_PAD, 128), BF16)
        xs_local[:DPC] = xb[c * DPC:(c + 1) * DPC]
        xTs = xs_local[np.minimum(co["perm"], DPC)].T.copy()
        in_maps.append({
            "xg": co["xg"].reshape(128, -1),
            "oh8": co["oh8"].reshape(128, -1),
            "ohT8": co["ohT8"].reshape(128, -1),
            "xTs": xTs,
            "Wqk": Wqk, "Ws": ws, "Wv": wv,
            "bqk1": bqk.reshape(1, 128), "bsv1": bsv.reshape(1, 128),
            "ident": ident,
        })
    return in_maps


def kernel(x, edge_index, Wq, bq, Wk, bk, Wv, bv, Ws, bs):
    from concourse import bass_utils

    xb = np.asarray(x, np.float32).astype(BF16)
    cores = _host_prep(edge_index, xb)
    in_maps = _make_in_maps(
        {"x": x, "Wq": Wq, "Wk": Wk, "Wv": Wv, "Ws": Ws,
         "bq": bq, "bs": bs, "bv": bv}, cores)
    nc = _get_nc()
    res = bass_utils.run_bass_kernel_spmd(nc, in_maps, core_ids=list(range(M_CORES)))
    out = np.zeros((N, 128), np.float32)
    for c in range(M_CORES):
        rows = res.results[c]["out"].astype(np.float32)
        p = cores[c]["perm"]
        valid = p < DPC
        out[c * DPC + p[valid]] = rows[valid]
    return out
